# revision 15
# baseline (speedup 1.0000x reference)
"""AttentionBlock (GroupNorm -> QKV 1x1 -> single-head attention -> out proj -> residual)
for x:(4,512,64,64) f32, distributed over 8 NeuronCores.

Sharding: data-parallel over batch, 2 cores per sample, each core owns 2048 of the
4096 query positions. Each core receives a column-ROTATED copy of its sample
(its local 2048 positions first) so the compiled program is identical on every
core (SPMD): Q/residual/output always address columns [0,2048); GroupNorm stats,
K and V use all 4096 columns (both are invariant to the column permutation).

Pipeline (fp8e4 DoubleRow matmuls throughout the hot path):
  1a) stats stream: one pass over x (f32; chunk DMAs split across the SP/Act
     hardware queues and the Pool software queue) into an SBUF-resident f32
     copy (XF32, also the residual source later -- no second HBM pass),
     computing bn_stats per channel row (DVE) while the Pool engine stages an
     fp8 copy of x in SBUF; group reduction -> per-channel affine (a, b) with
     xn = a*x + b, folded into the projection weights (W' = 16*W*diag(a),
     fp8; the x16 keeps the small weights out of fp8's subnormal range and
     is undone in the epilogues).
  1b) projections: K, Q, V^T as DoubleRow matmuls over the staged fp8 x,
     contracting 256 channels/pass -> fp8 outputs; epilogues split DVE/Act.
     Bias folds b' = b_w + W b run entirely on-chip as tiny f32r matmuls
     (N=2 moving dim; N=1 is invalid ISA) directly in channel layout -- no
     DRAM bounce. bk is dropped (softmax cancels per-i logit shifts); bv
     folds through the output projection (bo' = bo + Wo b'v, emitted AFTER
     the projection stream so nothing blocks on it).
  2) attention, stream-structured to decouple the engines: per query block,
     E[j,i] = exp(scale*K8^T Qsb - 2) runs as an E->exp pipeline (PE -> Act)
     whose 32 fp8 exp halves land in an SBUF buffer e8_all (the -2 shift
     keeps e8 < fp8e4's 240 max and cancels in the softmax ratio). The
     softmax denominator (ones-vector DR matmuls -> one PSUM bank) and
     O[c,i] += V8^T e8 (4 PSUM banks) then run as pure back-to-back PE
     streams with no cross-engine waits, with the NEXT block's E->exp
     stream interleaved into the PE queue (block 0's E stream interleaves
     into the projection s-loop). A single e8_all buffer is safe: the
     next block's exp for slot t write-after-read waits on this block's
     denominator+O reads of slot t, which run early in the window.
  3) out = Wo (O/denom) + bo' + x[:, :2048] -> y: fp8 DR matmuls into the
     just-freed O PSUM banks; ONE fused DVE op per 128-channel group
     (y = fps/16 + xres, with bo' pre-added into the SBUF residual), DMA out.

PSUM budget (8 banks): 3 eps ring + 4 O accumulators/out-proj + 1 denominator
(phase 1b: 4 projection ring + 1 bias-fold + 3 eps ring).
Weights are transposed host-side.
"""

import sys

sys.path.insert(0, "/opt/trn_rl_repo")

import numpy as np
from contextlib import ExitStack

import concourse.bass as bass
import concourse.tile as tile
from concourse import bacc, mybir
from concourse.masks import make_identity

F32 = mybir.dt.float32
F32R = mybir.dt.float32r
BF16 = mybir.dt.bfloat16
FP8 = mybir.dt.float8e4
DR = mybir.MatmulPerfMode.DoubleRow

C = 512          # channels
HW = 4096        # spatial positions per sample
L = 2048         # query positions per core
P = 128          # partitions
CO = C // P      # 4 channel chunks
NG = 32          # groups
GS = C // NG     # 16 channels per group
G_PER_CO = P // GS  # 8 groups per 128-partition chunk
EPS = 1e-6
SCALE = C ** -0.5
ESHIFT = -2.0    # exp(scale*logit - 2): keeps fp8e4 outputs < 240 (max logit ~6.2)
WSC = 16.0       # fp8 weight pre-scale (undone in the projection epilogues)
IB = 512         # query block
NIB = L // IB    # 4
NJ = HW // P     # 32 j-chunks
NT = NJ // 2     # 16 j-chunk pairs (DoubleRow)
NXC = HW // 512  # 8 x-stream chunks
B = 4            # batch
NCORES = 8

_cached = {}


def build_program(reps: int = 1, upto: str = "full"):
    nc = bacc.Bacc(None, target_bir_lowering=False)

    xf = nc.declare_dram_parameter("xf", [C, HW], F32, isOutput=False)
    wqt_d = nc.declare_dram_parameter("wqt", [C, C], F32R, isOutput=False)
    wkt_d = nc.declare_dram_parameter("wkt", [C, C], F32R, isOutput=False)
    wvt_d = nc.declare_dram_parameter("wvt", [C, C], F32R, isOutput=False)
    wot_d = nc.declare_dram_parameter("wot", [C, C], F32R, isOutput=False)
    bq_d = nc.declare_dram_parameter("bq", [C], F32, isOutput=False)
    bk_d = nc.declare_dram_parameter("bk", [C], F32, isOutput=False)
    bv_d = nc.declare_dram_parameter("bv", [C], F32, isOutput=False)
    bo_d = nc.declare_dram_parameter("bo", [C], F32, isOutput=False)
    gamma_d = nc.declare_dram_parameter("gamma", [C], F32, isOutput=False)
    beta_d = nc.declare_dram_parameter("beta", [C], F32, isOutput=False)
    y = nc.declare_dram_parameter("y", [C, L], F32, isOutput=True)

    # [c, j] -> [cp, coo, j] with c = coo*128 + cp
    xf_t = xf[:].rearrange("(coo cp) j -> cp coo j", cp=P)
    y_t = y[:].rearrange("(coo cp) i -> cp coo i", cp=P)

    with tile.TileContext(nc) as tc:
        for _rep in range(reps):
          with ExitStack() as ctx:
            consts = ctx.enter_context(tc.tile_pool(name="consts", bufs=1))
            big = ctx.enter_context(tc.tile_pool(name="big", bufs=1))
            esb = ctx.enter_context(tc.tile_pool(name="esb", bufs=2))

            ident = consts.tile([P, P], F32)
            make_identity(nc, ident)
            eps_t = consts.tile([CO, 1], F32)
            nc.vector.memset(eps_t, EPS)
            sc_row_f32 = consts.tile([1, P], F32)
            nc.vector.memset(sc_row_f32, 1.0)
            sc_row = consts.tile([1, P], F32R)
            nc.vector.tensor_copy(out=sc_row, in_=sc_row_f32)
            ones8_f32 = consts.tile([P, 32], F32)
            nc.vector.memset(ones8_f32, 1.0)
            # dual-fp8 Ldweights requires the row-pair dim stride to be a
            # multiple of 16 elements, so pad the pair stride to 16; the
            # denominator matmul uses 2 columns (producing identical rows)
            ones8_t = consts.tile([P, 2, 16], FP8)
            nc.vector.tensor_copy(
                out=ones8_t, in_=ones8_f32.rearrange("p (h o) -> p h o", o=16)
            )
            ones8 = ones8_t[:, :, 0:2]
            eshift_t = consts.tile([P, 1], F32)
            nc.vector.memset(eshift_t, ESHIFT)

            def load_chan_vec(name, dsrc):
                t = consts.tile([P, CO], F32, tag=name)
                nc.sync.dma_start(
                    out=t, in_=dsrc[:].rearrange("(coo cp) -> cp coo", cp=P)
                )
                return t

            # small vector loads are deprioritized so the x-chunk stream owns
            # the queues at t=0
            with tc.tile_wait_until(0.004):
                gamma_sb = load_chan_vec("gamma_sb", gamma_d)
                beta_sb = load_chan_vec("beta_sb", beta_d)
                # NOTE: bk is unused — adding bk shifts every logit column by
                # a per-i constant, which the softmax over j cancels exactly.
                bq_ch = load_chan_vec("bq_ch", bq_d)
                bv_ch = load_chan_vec("bv_ch", bv_d)
                bo_ch = load_chan_vec("bo_ch", bo_d)

            K8 = big.tile([P, CO, HW], FP8, tag="K8")
            V8 = big.tile([P, NJ, C], FP8, tag="V8")
            Qsb = big.tile([P, CO, L], FP8, tag="Qsb")
            XB = big.tile([P, CO, HW], FP8, tag="XB")
            XF32 = big.tile([P, CO, L], F32, tag="XF32")
            WoT = big.tile([P, CO, C], F32R, tag="WoT")
            Wo8 = big.tile([P, CO, C], FP8, tag="Wo8")
            WqT = big.tile([P, CO, C], F32R, tag="WqT")
            WkT = big.tile([P, CO, C], F32R, tag="WkT")
            WvT = big.tile([P, CO, C], F32R, tag="WvT")
            bqf = consts.tile([P, CO], F32, tag="bqf")
            bof = consts.tile([P, CO], F32, tag="bof")
            e8blks = {}

            # E + exp for pair t of query block ib: two DR matmuls through the
            # eps PSUM ring, exp on Act into the e8_all SBUF slot
            # E + exp for pair t of query block ib: two DR matmuls per
            # half through the single-bank eps ring, exp on Act per half
            def emit_E2(ib, t, e8blk, pepool):
                isl = slice(ib * IB, (ib + 1) * IB)
                for h in range(2):
                    jc = 2 * t + h
                    eps_ps = pepool.tile([P, IB], F32, tag="eps",
                                         name=f"eps{ib}_{jc}")
                    for g in range(2):
                        nc.tensor.matmul(
                            eps_ps,
                            lhsT=K8[:, 2 * g : 2 * g + 2,
                                    jc * P : (jc + 1) * P],
                            rhs=Qsb[:, 2 * g : 2 * g + 2, isl],
                            start=(g == 0), stop=(g == 1),
                            perf_mode=DR,
                        )
                    nc.scalar.activation(
                        out=e8blk[:, t, h, :], in_=eps_ps,
                        func=mybir.ActivationFunctionType.Exp,
                        scale=SCALE, bias=eshift_t,
                    )

            # ---------- Phase 1a: stats stream + fp8 x staging ----------
            with (
                tc.tile_pool(name="ph1", bufs=1) as ph1,
                tc.psum_pool(name="pp1", bufs=2) as pp1,
            ):
                    stats = ph1.tile([P, CO, NXC, 6], F32, tag="stats")
                    # x chunks split over the SP/Act hardware queues plus two
                    # on the Pool software queue; local chunks land in the
                    # persistent XF32 (residual source), remote chunks
                    # stream through a small ring (stats + fp8 staging only)
                    xq = [nc.sync, nc.scalar, nc.gpsimd, nc.sync, nc.scalar,
                          nc.gpsimd, nc.sync, nc.scalar]
                    for s in range(NXC):
                        xsl = slice(s * 512, (s + 1) * 512)
                        if s < L // 512:
                            xc = XF32[:, :, xsl]
                        else:
                            xc = ph1.tile([P, CO, 512], F32, tag="xc",
                                          name=f"xc{s}")
                        xq[s].dma_start(out=xc, in_=xf_t[:, :, xsl])
                        for coo in range(CO):
                            nc.vector.bn_stats(
                                out=stats[:, coo, s, :], in_=xc[:, coo, :]
                            )
                        nc.gpsimd.tensor_copy(out=XB[:, :, xsl], in_=xc)
                    # weight loads land during the stats stream, after the x
                    # chunks have drained their queues
                    with tc.tile_wait_until(0.008):
                        for eng, WT, wsrc in (
                            (nc.scalar, WkT, wkt_d), (nc.sync, WvT, wvt_d),
                            (nc.scalar, WqT, wqt_d), (nc.sync, WoT, wot_d),
                        ):
                            eng.dma_start(
                                out=WT,
                                in_=wsrc[:].rearrange(
                                    "(cio cp) co -> cp cio co", cp=P),
                            )
                    mv = ph1.tile([P, CO, 2], F32, tag="mv")
                    for coo in range(CO):
                        nc.vector.bn_aggr(out=mv[:, coo, :], in_=stats[:, coo, :, :])

                    # T_in cols 0:4 per-channel mean, 4:8 per-channel E[x^2]
                    T_in = ph1.tile([P, 8], F32, tag="T_in")
                    nc.vector.tensor_copy(T_in[:, 0:CO], mv[:, :, 0])
                    nc.vector.tensor_tensor(
                        out=T_in[:, CO : 2 * CO], in0=mv[:, :, 0], in1=mv[:, :, 0],
                        op=mybir.AluOpType.mult,
                    )
                    nc.vector.tensor_tensor(
                        out=T_in[:, CO : 2 * CO], in0=T_in[:, CO : 2 * CO],
                        in1=mv[:, :, 1], op=mybir.AluOpType.add,
                    )
                    tps = pp1.tile([8, P], F32, tag="wtp")
                    nc.tensor.transpose(tps, T_in, ident)
                    T_sb = ph1.tile([8, P], F32, tag="T_sb")
                    nc.vector.tensor_copy(T_sb, tps)
                    G = ph1.tile([8, G_PER_CO], F32, tag="G")
                    nc.vector.reduce_sum(
                        out=G, in_=T_sb.rearrange("p (g s) -> p g s", s=GS),
                        axis=mybir.AxisListType.X,
                    )
                    G2 = ph1.tile([CO, G_PER_CO], F32, tag="G2")
                    nc.scalar.dma_start(out=G2, in_=G[CO : 2 * CO, :])
                    mean_g = ph1.tile([CO, G_PER_CO], F32, tag="mean_g")
                    nc.scalar.mul(out=mean_g, in_=G[0:CO, :], mul=1.0 / GS)
                    var_g = ph1.tile([CO, G_PER_CO], F32, tag="var_g")
                    nc.vector.tensor_tensor(
                        out=var_g, in0=mean_g, in1=mean_g, op=mybir.AluOpType.mult
                    )
                    nc.vector.tensor_scalar(
                        out=G2, in0=G2, scalar1=1.0 / GS, scalar2=None,
                        op0=mybir.AluOpType.mult,
                    )
                    nc.vector.tensor_tensor(
                        out=var_g, in0=G2, in1=var_g, op=mybir.AluOpType.subtract
                    )
                    rstd_g = ph1.tile([CO, G_PER_CO], F32, tag="rstd_g")
                    nc.scalar.activation(
                        out=rstd_g, in_=var_g,
                        func=mybir.ActivationFunctionType.Sqrt,
                        bias=eps_t, scale=1.0,
                    )
                    nc.vector.reciprocal(out=rstd_g, in_=rstd_g)

                    # group -> channel broadcast along free, then PE transpose
                    Bm = ph1.tile([CO, P], F32, tag="Bm")
                    Br = ph1.tile([CO, P], F32, tag="Br")
                    for src, dst in ((mean_g, Bm), (rstd_g, Br)):
                        bc = bass.AP(
                            tensor=src.tensor, offset=src.offset,
                            ap=[src.ap[0], src.ap[1], [0, GS]],
                        )
                        nc.vector.tensor_copy(
                            dst.rearrange("p (g s) -> p g s", s=GS), bc
                        )
                    mean_ps = pp1.tile([P, CO], F32, tag="wtp", name="mean_ps")
                    rstd_ps = pp1.tile([P, CO], F32, tag="wtp", name="rstd_ps")
                    nc.tensor.transpose(mean_ps, Bm, ident[0:CO, 0:CO])
                    nc.tensor.transpose(rstd_ps, Br, ident[0:CO, 0:CO])
                    # a = gamma * rstd ; b = beta - mean * a   [128, 4] channel
                    a_ch = consts.tile([P, CO], F32, tag="a_ch")
                    b_ch = consts.tile([P, CO], F32R, tag="b_ch")
                    nc.vector.tensor_tensor(
                        out=a_ch, in0=gamma_sb, in1=rstd_ps, op=mybir.AluOpType.mult
                    )
                    nc.vector.tensor_tensor(
                        out=b_ch, in0=mean_ps, in1=a_ch, op=mybir.AluOpType.mult
                    )
                    nc.vector.tensor_tensor(
                        out=b_ch, in0=beta_sb, in1=b_ch, op=mybir.AluOpType.subtract
                    )

            if upto == "stats":
                tiny = consts.tile([P, CO], F32, tag="tiny",
                                   name=f"tiny{_rep}")
                nc.vector.tensor_scalar(
                    out=tiny, in0=a_ch, scalar1=2.0, scalar2=None,
                    op0=mybir.AluOpType.mult,
                )
                nc.sync.dma_start(out=y_t[:, 0, 0:CO], in_=tiny)
                continue

            # ---------- Phase 1b: weight prep + merged K/V/Q projection ----
            with (
                tc.tile_pool(name="phW", bufs=1) as phW,
                tc.psum_pool(name="ppb", bufs=1) as ppb,
                tc.psum_pool(name="ppmm", bufs=6) as ppmm,
            ):
                    # W'T[ci, co] = 16 * WT[ci, co] * a[ci], downcast to fp8e4
                    # (x16 keeps the ~N(0, 1/512) weights out of the fp8
                    # subnormal range; the projection epilogues divide by 16).
                    # K first so its projection matmuls can start earliest;
                    # prep is spread DVE/Pool so the first matmuls are quick.
                    a16 = consts.tile([P, CO], F32, tag="a16")
                    nc.vector.tensor_scalar(
                        out=a16, in0=a_ch, scalar1=float(WSC), scalar2=None,
                        op0=mybir.AluOpType.mult,
                    )
                    Wq8 = phW.tile([P, CO, C], FP8, tag="Wq8")
                    Wk8 = phW.tile([P, CO, C], FP8, tag="Wk8")
                    Wv8 = phW.tile([P, CO, C], FP8, tag="Wv8")
                    for cio in range(CO):
                        nc.vector.tensor_scalar_mul(
                            Wk8[:, cio, :], WkT[:, cio, :], a16[:, cio : cio + 1]
                        )
                    for cio in range(CO):
                        nc.gpsimd.tensor_scalar_mul(
                            Wv8[:, cio, :], WvT[:, cio, :], a16[:, cio : cio + 1]
                        )
                    for cio in range(CO):
                        nc.vector.tensor_scalar_mul(
                            Wq8[:, cio, :], WqT[:, cio, :], a16[:, cio : cio + 1]
                        )
                    for cio in range(CO):
                        nc.gpsimd.tensor_scalar_mul(
                            Wo8[:, cio, :], WoT[:, cio, :], float(WSC)
                        )

                    # on-chip bias folds, directly in channel layout:
                    # fold(W)[c] = sum_ci WT[ci, c] * b[ci] via tiny f32r
                    # matmuls (the rhs is duplicated to 2 columns since a
                    # 1-wide moving dim is invalid ISA for f32r)
                    b2 = phW.tile([P, CO, 2], F32R, tag="b2")
                    b_bc = bass.AP(
                        tensor=b_ch.tensor, offset=b_ch.offset,
                        ap=[b_ch.ap[0], b_ch.ap[1], [0, 2]],
                    )
                    nc.vector.tensor_copy(out=b2, in_=b_bc)

                    def fold(WT, rhs3, pname):
                        pf = ppb.tile([P, CO, 2], F32, tag="pfold",
                                      name=pname)
                        for coo in range(CO):
                            for cio in range(CO):
                                nc.tensor.matmul(
                                    pf[:, coo, :],
                                    lhsT=WT[:, cio, coo * P : (coo + 1) * P],
                                    rhs=rhs3[:, cio, :],
                                    start=(cio == 0), stop=(cio == CO - 1),
                                )
                        return pf

                    pf_v = fold(WvT, b2, "pf_v")
                    bvf2 = phW.tile([P, CO, 2], F32R, tag="bvf2")
                    bv_bc = bass.AP(
                        tensor=bv_ch.tensor, offset=bv_ch.offset,
                        ap=[bv_ch.ap[0], bv_ch.ap[1], [0, 2]],
                    )
                    nc.vector.tensor_tensor(
                        out=bvf2, in0=pf_v, in1=bv_bc, op=mybir.AluOpType.add
                    )
                    pf_q = fold(WqT, b2, "pf_q")
                    nc.vector.tensor_tensor(
                        out=bqf, in0=pf_q[:, :, 0], in1=bq_ch,
                        op=mybir.AluOpType.add,
                    )

                    # one pass over staged fp8 x: K, V^T and Q (local
                    # half), all as DoubleRow matmuls contracting 256
                    # channels/pass; epilogues split across DVE and Act
                    for s in range(NXC):
                        xsl = slice(s * 512, (s + 1) * 512)
                        for coo in range(CO):
                            pk = ppmm.tile([P, 512], F32, tag="pk",
                                           name=f"pk{s}_{coo}")
                            for g in range(2):
                                nc.tensor.matmul(
                                    pk,
                                    lhsT=Wk8[:, 2 * g : 2 * g + 2,
                                             coo * P : (coo + 1) * P],
                                    rhs=XB[:, 2 * g : 2 * g + 2, xsl],
                                    start=(g == 0), stop=(g == 1),
                                    perf_mode=DR,
                                )
                            if coo % 2 == 0:
                                nc.vector.tensor_scalar(
                                    out=K8[:, coo, xsl], in0=pk,
                                    scalar1=float(1.0 / WSC), scalar2=None,
                                    op0=mybir.AluOpType.mult,
                                )
                            else:
                                nc.scalar.mul(
                                    out=K8[:, coo, xsl], in_=pk,
                                    mul=float(1.0 / WSC),
                                )
                        for jsub in range(4):
                            pv = ppmm.tile([P, C], F32, tag="pk",
                                           name=f"pv{s}_{jsub}")
                            for g in range(2):
                                nc.tensor.matmul(
                                    pv,
                                    lhsT=XB[:, 2 * g : 2 * g + 2,
                                            s * 512 + jsub * P
                                            : s * 512 + (jsub + 1) * P],
                                    rhs=Wv8[:, 2 * g : 2 * g + 2, :],
                                    start=(g == 0), stop=(g == 1),
                                    perf_mode=DR,
                                )
                            if jsub % 2 == 0:
                                nc.scalar.mul(
                                    out=V8[:, s * 4 + jsub, :], in_=pv,
                                    mul=float(1.0 / WSC),
                                )
                            else:
                                nc.vector.tensor_scalar(
                                    out=V8[:, s * 4 + jsub, :], in0=pv,
                                    scalar1=float(1.0 / WSC), scalar2=None,
                                    op0=mybir.AluOpType.mult,
                                )
                        if s < L // 512:
                            for coo in range(CO):
                                pq = ppmm.tile([P, 512], F32, tag="pk",
                                               name=f"pq{s}_{coo}")
                                for g in range(2):
                                    nc.tensor.matmul(
                                        pq,
                                        lhsT=Wq8[:, 2 * g : 2 * g + 2,
                                                 coo * P : (coo + 1) * P],
                                        rhs=XB[:, 2 * g : 2 * g + 2, xsl],
                                        start=(g == 0), stop=(g == 1),
                                        perf_mode=DR,
                                    )
                                nc.vector.tensor_scalar(
                                    out=Qsb[:, coo, xsl], in0=pq,
                                    scalar1=float(1.0 / WSC),
                                    scalar2=bqf[:, coo : coo + 1],
                                    op0=mybir.AluOpType.mult,
                                    op1=mybir.AluOpType.add,
                                )

                    # bo' = bo + Wo b'v (softmax rows sum to 1, so the V bias
                    # can ride through the output projection); after the
                    # projections so nothing queues behind it
                    pf_o = fold(WoT, bvf2, "pf_o")
                    nc.vector.tensor_tensor(
                        out=bof, in0=pf_o[:, :, 0], in1=bo_ch,
                        op=mybir.AluOpType.add,
                    )

            if upto == "proj":
                tiny2 = consts.tile([P, CO], F32, tag="tiny2",
                                    name=f"tiny2{_rep}")
                nc.vector.tensor_copy(out=tiny2, in_=bof)
                nc.sync.dma_start(out=y_t[:, 0, 0:CO], in_=tiny2)
                continue

            # ---------- Phase 2: attention + output projection ----------
            with (
                tc.tile_pool(name="att", bufs=2) as att,
                tc.psum_pool(name="pe", bufs=3) as pe,
                tc.psum_pool(name="po", bufs=4) as po,
                tc.psum_pool(name="pd", bufs=1) as pd,
            ):
                def emit_outproj_coo(ib, O_sb, coo):
                    isl = slice(ib * IB, (ib + 1) * IB)
                    fps = po.tile([P, IB], F32, tag="ops",
                                  name=f"fps{ib}_{coo}")
                    for g in range(2):
                        nc.tensor.matmul(
                            fps,
                            lhsT=Wo8[:, 2 * g : 2 * g + 2,
                                     coo * P : (coo + 1) * P],
                            rhs=O_sb[:, 2 * g : 2 * g + 2, :],
                            start=(g == 0), stop=(g == 1), perf_mode=DR,
                        )
                    # fused epilogue (DVE: Pool can't read PSUM): undo Wo8's
                    # x16 pre-scale and add the (bo'-preadjusted) residual
                    ysb = att.tile([P, IB], F32, tag="ysb",
                                   name=f"ysb{ib}_{coo}")
                    nc.vector.scalar_tensor_tensor(
                        out=ysb, in0=fps, scalar=float(1.0 / WSC),
                        op0=mybir.AluOpType.mult,
                        in1=XF32[:, coo, isl], op1=mybir.AluOpType.add,
                    )
                    nc.sync.dma_start(out=y_t[:, coo, isl], in_=ysb)

                # A(0) runs solo first (Act-paced)
                e8blks[0] = esb.tile([P, NT, 2, IB], FP8, tag="e8blk",
                                     name="e8blk0")
                for t in range(NT):
                    emit_E2(0, t, e8blks[0], pe)

                if upto == "a0":
                    tiny3 = consts.tile([P, IB], F32, tag="tiny3",
                                        name=f"tiny3{_rep}")
                    nc.vector.tensor_copy(out=tiny3,
                                          in_=e8blks[0][:, NT - 1, 1, :])
                    nc.sync.dma_start(out=y_t[:, 0, 0:IB], in_=tiny3)
                    continue

                nwin = 1 if upto == "win0" else NIB
                for ib in range(nwin):
                    isl = slice(ib * IB, (ib + 1) * IB)
                    e8cur = e8blks.pop(ib)
                    if ib + 1 < NIB:
                        e8blks[ib + 1] = esb.tile([P, NT, 2, IB], FP8,
                                                  tag="e8blk",
                                                  name=f"e8blk{ib + 1}")
                    dps = pd.tile([2, IB], F32, tag="dps", name=f"dps{ib}")
                    ops = [
                        po.tile([P, IB], F32, tag="ops", name=f"ops{ib}_{i}")
                        for i in range(CO)
                    ]
                    # per-t consumer group [denom, O x4] + next block's
                    # E/exp pair: same-bank revisit distance stays >= 9
                    # matmuls, and the only Act-gated instructions are the
                    # E matmuls (2-pair eps-ring slack)
                    for t in range(NT):
                        if ib + 1 < NIB:
                            emit_E2(ib + 1, t, e8blks[ib + 1], pe)
                        nc.tensor.matmul(
                            dps, lhsT=ones8, rhs=e8cur[:, t, :, :],
                            start=(t == 0), stop=(t == NT - 1),
                            perf_mode=DR,
                        )
                        for cio in range(CO):
                            nc.tensor.matmul(
                                ops[cio],
                                lhsT=V8[:, 2 * t : 2 * t + 2,
                                        cio * P : (cio + 1) * P],
                                rhs=e8cur[:, t, :, :],
                                start=(t == 0), stop=(t == NT - 1),
                                perf_mode=DR,
                            )
                    recip = att.tile([1, IB], F32R, tag="recip",
                                     name=f"rc{ib}")
                    with nc.allow_low_precision(reason="f32r holds fp32 bits"):
                        nc.vector.reciprocal(out=recip, in_=dps[0:1, :])
                    # broadcast 1/denom across partitions via K=1 outer
                    # product; pre-add bo' into the residual on idle DVE
                    bct = pe.tile([P, IB], F32, tag="eps", name=f"bc{ib}")
                    nc.tensor.matmul(
                        bct, lhsT=sc_row, rhs=recip, start=True, stop=True,
                    )
                    bcast_sb = att.tile([P, IB], F32, tag="bcast",
                                        name=f"bs{ib}")
                    nc.vector.tensor_copy(out=bcast_sb, in_=bct)
                    for coo in range(CO):
                        nc.vector.tensor_scalar(
                            out=XF32[:, coo, isl], in0=XF32[:, coo, isl],
                            scalar1=bof[:, coo : coo + 1], scalar2=None,
                            op0=mybir.AluOpType.add,
                        )
                    O_sb = att.tile([P, CO, IB], FP8, tag="O_sb",
                                    name=f"osb{ib}")
                    for cio in range(CO):
                        nc.vector.tensor_tensor(
                            out=O_sb[:, cio, :], in0=ops[cio], in1=bcast_sb,
                            op=mybir.AluOpType.mult,
                        )
                    for coo in range(CO):
                        emit_outproj_coo(ib, O_sb, coo)

    nc.compile()
    return nc


def get_program(reps: int = 1, upto: str = "full"):
    key = f"nc{reps}_{upto}"
    if key not in _cached:
        _cached[key] = build_program(reps, upto)
    return _cached[key]


def make_in_maps(inputs):
    x = np.asarray(inputs["x"], np.float32).reshape(B, C, HW)
    common = {
        k: np.ascontiguousarray(np.asarray(inputs[k], np.float32))
        for k in ("bq", "bk", "bv", "bo", "gamma", "beta")
    }
    for k in ("wq", "wk", "wv", "wo"):
        common[k + "t"] = np.ascontiguousarray(np.asarray(inputs[k], np.float32).T)
    in_maps = []
    for core in range(NCORES):
        b, h = core // 2, core % 2
        loc = x[b][:, h * L : (h + 1) * L]
        oth = x[b][:, (1 - h) * L : (2 - h) * L]
        xf_rot = np.ascontiguousarray(np.concatenate([loc, oth], axis=1))
        m = dict(common)
        m["xf"] = xf_rot
        in_maps.append(m)
    return in_maps


def kernel(**inputs) -> np.ndarray:
    from concourse.bass_utils import run_bass_kernel_spmd

    nc = get_program()
    in_maps = make_in_maps(inputs)
    res = run_bass_kernel_spmd(nc, in_maps, list(range(NCORES)))
    out = np.empty((B, C, HW), np.float32)
    for core in range(NCORES):
        b, h = core // 2, core % 2
        out[b][:, h * L : (h + 1) * L] = res.results[core]["y"]
    return out.reshape(B, C, 64, 64)


# revision 17
# speedup vs baseline: 1.0302x; 1.0302x over previous
"""AttentionBlock (GroupNorm -> QKV 1x1 -> single-head attention -> out proj -> residual)
for x:(4,512,64,64) f32, distributed over 8 NeuronCores.

Sharding: data-parallel over batch, 2 cores per sample, each core owns 2048 of the
4096 query positions. Each core receives a column-ROTATED copy of its sample
(its local 2048 positions first) so the compiled program is identical on every
core (SPMD): Q/residual/output always address columns [0,2048); GroupNorm stats,
K and V use all 4096 columns (both are invariant to the column permutation).

Pipeline (fp8e4 DoubleRow matmuls throughout the hot path):
  1a) stats stream: one pass over x (f32; chunk DMAs split across the SP/Act
     hardware queues and the Pool software queue) into an SBUF-resident f32
     copy (XF32, also the residual source later -- no second HBM pass),
     computing bn_stats per channel row (DVE) while the Pool engine stages an
     fp8 copy of x in SBUF; group reduction -> per-channel affine (a, b) with
     xn = a*x + b, folded into the projection weights (W' = 16*W*diag(a),
     fp8; the x16 keeps the small weights out of fp8's subnormal range and
     is undone in the epilogues).
  1b) projections: K, Q, V^T as DoubleRow matmuls over the staged fp8 x,
     contracting 256 channels/pass -> fp8 outputs; epilogues split DVE/Act.
     Bias folds b' = b_w + W b run entirely on-chip as tiny f32r matmuls
     (N=2 moving dim; N=1 is invalid ISA) directly in channel layout -- no
     DRAM bounce. bk is dropped (softmax cancels per-i logit shifts); bv
     folds through the output projection (bo' = bo + Wo b'v, emitted AFTER
     the projection stream so nothing blocks on it).
  2) attention, stream-structured to decouple the engines: per query block,
     E[j,i] = exp(scale*K8^T Qsb - 2) runs as an E->exp pipeline (PE -> Act)
     whose 32 fp8 exp halves land in an SBUF buffer e8_all (the -2 shift
     keeps e8 < fp8e4's 240 max and cancels in the softmax ratio). The
     softmax denominator (ones-vector DR matmuls -> one PSUM bank) and
     O[c,i] += V8^T e8 (4 PSUM banks) then run as pure back-to-back PE
     streams with no cross-engine waits, with the NEXT block's E->exp
     stream interleaved into the PE queue (block 0's E stream interleaves
     into the projection s-loop). A single e8_all buffer is safe: the
     next block's exp for slot t write-after-read waits on this block's
     denominator+O reads of slot t, which run early in the window.
  3) out = Wo (O/denom) + bo' + x[:, :2048] -> y: fp8 DR matmuls into the
     just-freed O PSUM banks; ONE fused DVE op per 128-channel group
     (y = fps/16 + xres, with bo' pre-added into the SBUF residual), DMA out.

PSUM budget (8 banks): 3 eps ring + 4 O accumulators/out-proj + 1 denominator
(phase 1b: 4 projection ring + 1 bias-fold + 3 eps ring).
Weights are transposed host-side.
"""

import sys

sys.path.insert(0, "/opt/trn_rl_repo")

import numpy as np
from contextlib import ExitStack

import concourse.bass as bass
import concourse.tile as tile
from concourse import bacc, mybir
from concourse.masks import make_identity

F32 = mybir.dt.float32
F32R = mybir.dt.float32r
BF16 = mybir.dt.bfloat16
FP8 = mybir.dt.float8e4
DR = mybir.MatmulPerfMode.DoubleRow

C = 512          # channels
HW = 4096        # spatial positions per sample
L = 2048         # query positions per core
P = 128          # partitions
CO = C // P      # 4 channel chunks
NG = 32          # groups
GS = C // NG     # 16 channels per group
G_PER_CO = P // GS  # 8 groups per 128-partition chunk
EPS = 1e-6
SCALE = C ** -0.5
ESHIFT = -2.0    # exp(scale*logit - 2): keeps fp8e4 outputs < 240 (max logit ~6.2)
WSC = 16.0       # fp8 weight pre-scale (undone in the projection epilogues)
IB = 512         # query block
NIB = L // IB    # 4
NJ = HW // P     # 32 j-chunks
NT = NJ // 2     # 16 j-chunk pairs (DoubleRow)
NXC = HW // 512  # 8 x-stream chunks
B = 4            # batch
NCORES = 8

_cached = {}


def build_program(reps: int = 1, upto: str = "full"):
    nc = bacc.Bacc(None, target_bir_lowering=False)

    xf = nc.declare_dram_parameter("xf", [C, HW], F32, isOutput=False)
    wqt_d = nc.declare_dram_parameter("wqt", [C, C], F32R, isOutput=False)
    wkt_d = nc.declare_dram_parameter("wkt", [C, C], F32R, isOutput=False)
    wvt_d = nc.declare_dram_parameter("wvt", [C, C], F32R, isOutput=False)
    wot_d = nc.declare_dram_parameter("wot", [C, C], F32R, isOutput=False)
    bq_d = nc.declare_dram_parameter("bq", [C], F32, isOutput=False)
    bk_d = nc.declare_dram_parameter("bk", [C], F32, isOutput=False)
    bv_d = nc.declare_dram_parameter("bv", [C], F32, isOutput=False)
    bo_d = nc.declare_dram_parameter("bo", [C], F32, isOutput=False)
    gamma_d = nc.declare_dram_parameter("gamma", [C], F32, isOutput=False)
    beta_d = nc.declare_dram_parameter("beta", [C], F32, isOutput=False)
    y = nc.declare_dram_parameter("y", [C, L], F32, isOutput=True)

    # [c, j] -> [cp, coo, j] with c = coo*128 + cp
    xf_t = xf[:].rearrange("(coo cp) j -> cp coo j", cp=P)
    y_t = y[:].rearrange("(coo cp) i -> cp coo i", cp=P)

    with tile.TileContext(nc) as tc:
        for _rep in range(reps):
          with ExitStack() as ctx:
            consts = ctx.enter_context(tc.tile_pool(name="consts", bufs=1))
            big = ctx.enter_context(tc.tile_pool(name="big", bufs=1))
            esb = ctx.enter_context(tc.tile_pool(name="esb", bufs=2))

            ident = consts.tile([P, P], F32)
            make_identity(nc, ident)
            eps_t = consts.tile([CO, 1], F32)
            nc.vector.memset(eps_t, EPS)
            sc_row_f32 = consts.tile([1, P], F32)
            nc.vector.memset(sc_row_f32, 1.0)
            sc_row = consts.tile([1, P], F32R)
            nc.vector.tensor_copy(out=sc_row, in_=sc_row_f32)
            ones8_f32 = consts.tile([P, 32], F32)
            nc.vector.memset(ones8_f32, 1.0)
            # dual-fp8 Ldweights requires the row-pair dim stride to be a
            # multiple of 16 elements, so pad the pair stride to 16; the
            # denominator matmul uses 2 columns (producing identical rows)
            ones8_t = consts.tile([P, 2, 16], FP8)
            nc.vector.tensor_copy(
                out=ones8_t, in_=ones8_f32.rearrange("p (h o) -> p h o", o=16)
            )
            ones8 = ones8_t[:, :, 0:2]
            eshift_t = consts.tile([P, 1], F32)
            nc.vector.memset(eshift_t, ESHIFT)

            def load_chan_vec(name, dsrc):
                t = consts.tile([P, CO], F32, tag=name)
                nc.sync.dma_start(
                    out=t, in_=dsrc[:].rearrange("(coo cp) -> cp coo", cp=P)
                )
                return t

            # small vector loads are deprioritized so the x-chunk stream owns
            # the queues at t=0
            with tc.tile_wait_until(0.004):
                gamma_sb = load_chan_vec("gamma_sb", gamma_d)
                beta_sb = load_chan_vec("beta_sb", beta_d)
                # NOTE: bk is unused — adding bk shifts every logit column by
                # a per-i constant, which the softmax over j cancels exactly.
                bq_ch = load_chan_vec("bq_ch", bq_d)
                bv_ch = load_chan_vec("bv_ch", bv_d)
                bo_ch = load_chan_vec("bo_ch", bo_d)

            K8 = big.tile([P, CO, HW], FP8, tag="K8")
            V8 = big.tile([P, NJ, C], FP8, tag="V8")
            Qsb = big.tile([P, CO, L], FP8, tag="Qsb")
            XB = big.tile([P, CO, HW], FP8, tag="XB")
            XF32 = big.tile([P, CO, L], F32, tag="XF32")
            WoT = big.tile([P, CO, C], F32R, tag="WoT")
            Wo8 = big.tile([P, CO, C], FP8, tag="Wo8")
            WqT = big.tile([P, CO, C], F32R, tag="WqT")
            WkT = big.tile([P, CO, C], F32R, tag="WkT")
            WvT = big.tile([P, CO, C], F32R, tag="WvT")
            bqf = consts.tile([P, CO], F32, tag="bqf")
            bof = consts.tile([P, CO], F32, tag="bof")
            e8blks = {}

            # E + exp for pair t of query block ib: two DR matmuls through the
            # eps PSUM ring, exp on Act into the e8_all SBUF slot
            # E + exp for pair t of query block ib: two DR matmuls per
            # half through the single-bank eps ring, exp on Act per half
            def emit_E2(ib, t, e8blk, pepool):
                isl = slice(ib * IB, (ib + 1) * IB)
                for h in range(2):
                    jc = 2 * t + h
                    eps_ps = pepool.tile([P, IB], F32, tag="eps",
                                         name=f"eps{ib}_{jc}")
                    for g in range(2):
                        nc.tensor.matmul(
                            eps_ps,
                            lhsT=K8[:, 2 * g : 2 * g + 2,
                                    jc * P : (jc + 1) * P],
                            rhs=Qsb[:, 2 * g : 2 * g + 2, isl],
                            start=(g == 0), stop=(g == 1),
                            perf_mode=DR,
                        )
                    nc.scalar.activation(
                        out=e8blk[:, t, h, :], in_=eps_ps,
                        func=mybir.ActivationFunctionType.Exp,
                        scale=SCALE, bias=eshift_t,
                    )

            # ---------- Phase 1a: stats stream + fp8 x staging ----------
            with (
                tc.tile_pool(name="ph1", bufs=1) as ph1,
                tc.psum_pool(name="pp1", bufs=2) as pp1,
            ):
                    stats = ph1.tile([P, CO, NXC, 6], F32, tag="stats")
                    # x chunks split over the SP/Act hardware queues plus two
                    # on the Pool software queue; local chunks land in the
                    # persistent XF32 (residual source), remote chunks
                    # stream through a small ring (stats + fp8 staging only)
                    xq = [nc.sync, nc.scalar, nc.gpsimd, nc.sync, nc.scalar,
                          nc.gpsimd, nc.sync, nc.scalar]
                    for s in range(NXC):
                        xsl = slice(s * 512, (s + 1) * 512)
                        if s < L // 512:
                            xc = XF32[:, :, xsl]
                        else:
                            xc = ph1.tile([P, CO, 512], F32, tag="xc",
                                          name=f"xc{s}")
                        xq[s].dma_start(out=xc, in_=xf_t[:, :, xsl])
                        for coo in range(CO):
                            nc.vector.bn_stats(
                                out=stats[:, coo, s, :], in_=xc[:, coo, :]
                            )
                        nc.gpsimd.tensor_copy(out=XB[:, :, xsl], in_=xc)
                    # weight loads land during the stats stream, after the x
                    # chunks have drained their queues
                    with tc.tile_wait_until(0.008):
                        for eng, WT, wsrc in (
                            (nc.scalar, WkT, wkt_d), (nc.sync, WvT, wvt_d),
                            (nc.scalar, WqT, wqt_d), (nc.sync, WoT, wot_d),
                        ):
                            eng.dma_start(
                                out=WT,
                                in_=wsrc[:].rearrange(
                                    "(cio cp) co -> cp cio co", cp=P),
                            )
                    mv = ph1.tile([P, CO, 2], F32, tag="mv")
                    for coo in range(CO):
                        nc.vector.bn_aggr(out=mv[:, coo, :], in_=stats[:, coo, :, :])

                    # T_in cols 0:4 per-channel mean, 4:8 per-channel E[x^2]
                    T_in = ph1.tile([P, 8], F32, tag="T_in")
                    nc.vector.tensor_copy(T_in[:, 0:CO], mv[:, :, 0])
                    nc.vector.tensor_tensor(
                        out=T_in[:, CO : 2 * CO], in0=mv[:, :, 0], in1=mv[:, :, 0],
                        op=mybir.AluOpType.mult,
                    )
                    nc.vector.tensor_tensor(
                        out=T_in[:, CO : 2 * CO], in0=T_in[:, CO : 2 * CO],
                        in1=mv[:, :, 1], op=mybir.AluOpType.add,
                    )
                    tps = pp1.tile([8, P], F32, tag="wtp")
                    nc.tensor.transpose(tps, T_in, ident)
                    T_sb = ph1.tile([8, P], F32, tag="T_sb")
                    nc.vector.tensor_copy(T_sb, tps)
                    G = ph1.tile([8, G_PER_CO], F32, tag="G")
                    nc.vector.reduce_sum(
                        out=G, in_=T_sb.rearrange("p (g s) -> p g s", s=GS),
                        axis=mybir.AxisListType.X,
                    )
                    G2 = ph1.tile([CO, G_PER_CO], F32, tag="G2")
                    nc.scalar.dma_start(out=G2, in_=G[CO : 2 * CO, :])
                    mean_g = ph1.tile([CO, G_PER_CO], F32, tag="mean_g")
                    nc.scalar.mul(out=mean_g, in_=G[0:CO, :], mul=1.0 / GS)
                    var_g = ph1.tile([CO, G_PER_CO], F32, tag="var_g")
                    nc.vector.tensor_tensor(
                        out=var_g, in0=mean_g, in1=mean_g, op=mybir.AluOpType.mult
                    )
                    nc.vector.tensor_scalar(
                        out=G2, in0=G2, scalar1=1.0 / GS, scalar2=None,
                        op0=mybir.AluOpType.mult,
                    )
                    nc.vector.tensor_tensor(
                        out=var_g, in0=G2, in1=var_g, op=mybir.AluOpType.subtract
                    )
                    rstd_g = ph1.tile([CO, G_PER_CO], F32, tag="rstd_g")
                    nc.scalar.activation(
                        out=rstd_g, in_=var_g,
                        func=mybir.ActivationFunctionType.Sqrt,
                        bias=eps_t, scale=1.0,
                    )
                    nc.vector.reciprocal(out=rstd_g, in_=rstd_g)

                    # group -> channel broadcast along free, then PE transpose
                    Bm = ph1.tile([CO, P], F32, tag="Bm")
                    Br = ph1.tile([CO, P], F32, tag="Br")
                    for src, dst in ((mean_g, Bm), (rstd_g, Br)):
                        bc = bass.AP(
                            tensor=src.tensor, offset=src.offset,
                            ap=[src.ap[0], src.ap[1], [0, GS]],
                        )
                        nc.vector.tensor_copy(
                            dst.rearrange("p (g s) -> p g s", s=GS), bc
                        )
                    mean_ps = pp1.tile([P, CO], F32, tag="wtp", name="mean_ps")
                    rstd_ps = pp1.tile([P, CO], F32, tag="wtp", name="rstd_ps")
                    nc.tensor.transpose(mean_ps, Bm, ident[0:CO, 0:CO])
                    nc.tensor.transpose(rstd_ps, Br, ident[0:CO, 0:CO])
                    # a = gamma * rstd ; b = beta - mean * a   [128, 4] channel
                    a_ch = consts.tile([P, CO], F32, tag="a_ch")
                    b_ch = consts.tile([P, CO], F32R, tag="b_ch")
                    nc.vector.tensor_tensor(
                        out=a_ch, in0=gamma_sb, in1=rstd_ps, op=mybir.AluOpType.mult
                    )
                    nc.vector.tensor_tensor(
                        out=b_ch, in0=mean_ps, in1=a_ch, op=mybir.AluOpType.mult
                    )
                    nc.vector.tensor_tensor(
                        out=b_ch, in0=beta_sb, in1=b_ch, op=mybir.AluOpType.subtract
                    )

            if upto == "stats":
                tiny = consts.tile([P, CO], F32, tag="tiny",
                                   name=f"tiny{_rep}")
                nc.vector.tensor_scalar(
                    out=tiny, in0=a_ch, scalar1=2.0, scalar2=None,
                    op0=mybir.AluOpType.mult,
                )
                nc.sync.dma_start(out=y_t[:, 0, 0:CO], in_=tiny)
                continue

            # ---------- Phase 1b: weight prep + merged K/V/Q projection ----
            with (
                tc.tile_pool(name="phW", bufs=1) as phW,
                tc.psum_pool(name="ppb", bufs=1) as ppb,
                tc.psum_pool(name="ppmm", bufs=6) as ppmm,
            ):
                    # W'T[ci, co] = 16 * WT[ci, co] * a[ci], downcast to fp8e4
                    # (x16 keeps the ~N(0, 1/512) weights out of the fp8
                    # subnormal range; the projection epilogues divide by 16).
                    # K first so its projection matmuls can start earliest;
                    # prep is spread DVE/Pool so the first matmuls are quick.
                    a16 = consts.tile([P, CO], F32, tag="a16")
                    nc.vector.tensor_scalar(
                        out=a16, in0=a_ch, scalar1=float(WSC), scalar2=None,
                        op0=mybir.AluOpType.mult,
                    )
                    Wq8 = phW.tile([P, CO, C], FP8, tag="Wq8")
                    Wk8 = phW.tile([P, CO, C], FP8, tag="Wk8")
                    Wv8 = phW.tile([P, CO, C], FP8, tag="Wv8")
                    for cio in range(CO):
                        nc.vector.tensor_scalar_mul(
                            Wk8[:, cio, :], WkT[:, cio, :], a16[:, cio : cio + 1]
                        )
                    for cio in range(CO):
                        nc.gpsimd.tensor_scalar_mul(
                            Wv8[:, cio, :], WvT[:, cio, :], a16[:, cio : cio + 1]
                        )
                    for cio in range(CO):
                        nc.vector.tensor_scalar_mul(
                            Wq8[:, cio, :], WqT[:, cio, :], a16[:, cio : cio + 1]
                        )
                    for cio in range(CO):
                        nc.gpsimd.tensor_scalar_mul(
                            Wo8[:, cio, :], WoT[:, cio, :], float(WSC)
                        )

                    # on-chip bias folds, directly in channel layout:
                    # fold(W)[c] = sum_ci WT[ci, c] * b[ci] via tiny f32r
                    # matmuls (the rhs is duplicated to 2 columns since a
                    # 1-wide moving dim is invalid ISA for f32r)
                    b2 = phW.tile([P, CO, 2], F32R, tag="b2")
                    b_bc = bass.AP(
                        tensor=b_ch.tensor, offset=b_ch.offset,
                        ap=[b_ch.ap[0], b_ch.ap[1], [0, 2]],
                    )
                    nc.vector.tensor_copy(out=b2, in_=b_bc)

                    def fold(WT, rhs3, pname):
                        pf = ppb.tile([P, CO, 2], F32, tag="pfold",
                                      name=pname)
                        for coo in range(CO):
                            for cio in range(CO):
                                nc.tensor.matmul(
                                    pf[:, coo, :],
                                    lhsT=WT[:, cio, coo * P : (coo + 1) * P],
                                    rhs=rhs3[:, cio, :],
                                    start=(cio == 0), stop=(cio == CO - 1),
                                )
                        return pf

                    pf_v = fold(WvT, b2, "pf_v")
                    bvf2 = phW.tile([P, CO, 2], F32R, tag="bvf2")
                    bv_bc = bass.AP(
                        tensor=bv_ch.tensor, offset=bv_ch.offset,
                        ap=[bv_ch.ap[0], bv_ch.ap[1], [0, 2]],
                    )
                    nc.vector.tensor_tensor(
                        out=bvf2, in0=pf_v, in1=bv_bc, op=mybir.AluOpType.add
                    )
                    pf_q = fold(WqT, b2, "pf_q")
                    nc.vector.tensor_tensor(
                        out=bqf, in0=pf_q[:, :, 0], in1=bq_ch,
                        op=mybir.AluOpType.add,
                    )

                    # one pass over staged fp8 x: K, V^T and Q (local
                    # half), all as DoubleRow matmuls contracting 256
                    # channels/pass; epilogues split across DVE and Act
                    for s in range(NXC):
                        xsl = slice(s * 512, (s + 1) * 512)
                        for coo in range(CO):
                            pk = ppmm.tile([P, 512], F32, tag="pk",
                                           name=f"pk{s}_{coo}")
                            for g in range(2):
                                nc.tensor.matmul(
                                    pk,
                                    lhsT=Wk8[:, 2 * g : 2 * g + 2,
                                             coo * P : (coo + 1) * P],
                                    rhs=XB[:, 2 * g : 2 * g + 2, xsl],
                                    start=(g == 0), stop=(g == 1),
                                    perf_mode=DR,
                                )
                            if coo % 2 == 0:
                                nc.vector.tensor_scalar(
                                    out=K8[:, coo, xsl], in0=pk,
                                    scalar1=float(1.0 / WSC), scalar2=None,
                                    op0=mybir.AluOpType.mult,
                                )
                            else:
                                nc.scalar.mul(
                                    out=K8[:, coo, xsl], in_=pk,
                                    mul=float(1.0 / WSC),
                                )
                        for jsub in range(4):
                            pv = ppmm.tile([P, C], F32, tag="pk",
                                           name=f"pv{s}_{jsub}")
                            for g in range(2):
                                nc.tensor.matmul(
                                    pv,
                                    lhsT=XB[:, 2 * g : 2 * g + 2,
                                            s * 512 + jsub * P
                                            : s * 512 + (jsub + 1) * P],
                                    rhs=Wv8[:, 2 * g : 2 * g + 2, :],
                                    start=(g == 0), stop=(g == 1),
                                    perf_mode=DR,
                                )
                            if jsub % 2 == 0:
                                nc.scalar.mul(
                                    out=V8[:, s * 4 + jsub, :], in_=pv,
                                    mul=float(1.0 / WSC),
                                )
                            else:
                                nc.vector.tensor_scalar(
                                    out=V8[:, s * 4 + jsub, :], in0=pv,
                                    scalar1=float(1.0 / WSC), scalar2=None,
                                    op0=mybir.AluOpType.mult,
                                )
                        if s < L // 512:
                            for coo in range(CO):
                                pq = ppmm.tile([P, 512], F32, tag="pk",
                                               name=f"pq{s}_{coo}")
                                for g in range(2):
                                    nc.tensor.matmul(
                                        pq,
                                        lhsT=Wq8[:, 2 * g : 2 * g + 2,
                                                 coo * P : (coo + 1) * P],
                                        rhs=XB[:, 2 * g : 2 * g + 2, xsl],
                                        start=(g == 0), stop=(g == 1),
                                        perf_mode=DR,
                                    )
                                nc.vector.tensor_scalar(
                                    out=Qsb[:, coo, xsl], in0=pq,
                                    scalar1=float(1.0 / WSC),
                                    scalar2=bqf[:, coo : coo + 1],
                                    op0=mybir.AluOpType.mult,
                                    op1=mybir.AluOpType.add,
                                )

                    # bo' = bo + Wo b'v (softmax rows sum to 1, so the V bias
                    # can ride through the output projection); after the
                    # projections so nothing queues behind it
                    pf_o = fold(WoT, bvf2, "pf_o")
                    nc.vector.tensor_tensor(
                        out=bof, in0=pf_o[:, :, 0], in1=bo_ch,
                        op=mybir.AluOpType.add,
                    )

            if upto == "proj":
                tiny2 = consts.tile([P, CO], F32, tag="tiny2",
                                    name=f"tiny2{_rep}")
                nc.vector.tensor_copy(out=tiny2, in_=bof)
                nc.sync.dma_start(out=y_t[:, 0, 0:CO], in_=tiny2)
                continue

            # A(0) runs solo first (Act-paced). The rest of PSUM is idle
            # here, so use a dedicated DEEP eps ring: with only 3 slots the
            # in-order engines degenerate into a lockstep that pays the full
            # cross-engine semaphore round trip per half.
            e8blks[0] = esb.tile([P, NT, 2, IB], FP8, tag="e8blk",
                                 name="e8blk0")
            with tc.psum_pool(name="peA", bufs=8) as peA:
                for t in range(NT):
                    emit_E2(0, t, e8blks[0], peA)

            if upto == "a0":
                tiny3 = consts.tile([P, IB], F32, tag="tiny3",
                                    name=f"tiny3{_rep}")
                nc.vector.tensor_copy(out=tiny3,
                                      in_=e8blks[0][:, NT - 1, 1, :])
                nc.sync.dma_start(out=y_t[:, 0, 0:IB], in_=tiny3)
                continue

            # ---------- Phase 2: attention + output projection ----------
            with (
                tc.tile_pool(name="att", bufs=2) as att,
                tc.psum_pool(name="pe", bufs=3) as pe,
                tc.psum_pool(name="po", bufs=4) as po,
                tc.psum_pool(name="pd", bufs=1) as pd,
            ):
                def emit_outproj_coo(ib, O_sb, coo):
                    isl = slice(ib * IB, (ib + 1) * IB)
                    fps = po.tile([P, IB], F32, tag="ops",
                                  name=f"fps{ib}_{coo}")
                    for g in range(2):
                        nc.tensor.matmul(
                            fps,
                            lhsT=Wo8[:, 2 * g : 2 * g + 2,
                                     coo * P : (coo + 1) * P],
                            rhs=O_sb[:, 2 * g : 2 * g + 2, :],
                            start=(g == 0), stop=(g == 1), perf_mode=DR,
                        )
                    # fused epilogue (DVE: Pool can't read PSUM): undo Wo8's
                    # x16 pre-scale and add the (bo'-preadjusted) residual
                    ysb = att.tile([P, IB], F32, tag="ysb",
                                   name=f"ysb{ib}_{coo}")
                    nc.vector.scalar_tensor_tensor(
                        out=ysb, in0=fps, scalar=float(1.0 / WSC),
                        op0=mybir.AluOpType.mult,
                        in1=XF32[:, coo, isl], op1=mybir.AluOpType.add,
                    )
                    nc.sync.dma_start(out=y_t[:, coo, isl], in_=ysb)

                nwin = (1 if upto == "win0" else 2 if upto == "win1"
                        else NIB)
                for ib in range(nwin):
                    isl = slice(ib * IB, (ib + 1) * IB)
                    e8cur = e8blks.pop(ib)
                    if ib + 1 < NIB:
                        e8blks[ib + 1] = esb.tile([P, NT, 2, IB], FP8,
                                                  tag="e8blk",
                                                  name=f"e8blk{ib + 1}")
                    dps = pd.tile([2, IB], F32, tag="dps", name=f"dps{ib}")
                    ops = [
                        po.tile([P, IB], F32, tag="ops", name=f"ops{ib}_{i}")
                        for i in range(CO)
                    ]
                    # per-t consumer group [denom, O x4] + next block's
                    # E/exp pair: same-bank revisit distance stays >= 9
                    # matmuls, and the only Act-gated instructions are the
                    # E matmuls (2-pair eps-ring slack)
                    for t in range(NT):
                        if ib + 1 < NIB:
                            emit_E2(ib + 1, t, e8blks[ib + 1], pe)
                        nc.tensor.matmul(
                            dps, lhsT=ones8, rhs=e8cur[:, t, :, :],
                            start=(t == 0), stop=(t == NT - 1),
                            perf_mode=DR,
                        )
                        for cio in range(CO):
                            nc.tensor.matmul(
                                ops[cio],
                                lhsT=V8[:, 2 * t : 2 * t + 2,
                                        cio * P : (cio + 1) * P],
                                rhs=e8cur[:, t, :, :],
                                start=(t == 0), stop=(t == NT - 1),
                                perf_mode=DR,
                            )
                    recip = att.tile([1, IB], F32R, tag="recip",
                                     name=f"rc{ib}")
                    with nc.allow_low_precision(reason="f32r holds fp32 bits"):
                        nc.vector.reciprocal(out=recip, in_=dps[0:1, :])
                    # broadcast 1/denom across partitions via K=1 outer
                    # product; pre-add bo' into the residual on idle DVE
                    bct = pe.tile([P, IB], F32, tag="eps", name=f"bc{ib}")
                    nc.tensor.matmul(
                        bct, lhsT=sc_row, rhs=recip, start=True, stop=True,
                    )
                    bcast_sb = att.tile([P, IB], F32, tag="bcast",
                                        name=f"bs{ib}")
                    nc.vector.tensor_copy(out=bcast_sb, in_=bct)
                    for coo in range(CO):
                        nc.vector.tensor_scalar(
                            out=XF32[:, coo, isl], in0=XF32[:, coo, isl],
                            scalar1=bof[:, coo : coo + 1], scalar2=None,
                            op0=mybir.AluOpType.add,
                        )
                    O_sb = att.tile([P, CO, IB], FP8, tag="O_sb",
                                    name=f"osb{ib}")
                    for cio in range(CO):
                        nc.vector.tensor_tensor(
                            out=O_sb[:, cio, :], in0=ops[cio], in1=bcast_sb,
                            op=mybir.AluOpType.mult,
                        )
                    for coo in range(CO):
                        emit_outproj_coo(ib, O_sb, coo)

    nc.compile()
    return nc


def get_program(reps: int = 1, upto: str = "full"):
    key = f"nc{reps}_{upto}"
    if key not in _cached:
        _cached[key] = build_program(reps, upto)
    return _cached[key]


def make_in_maps(inputs):
    x = np.asarray(inputs["x"], np.float32).reshape(B, C, HW)
    common = {
        k: np.ascontiguousarray(np.asarray(inputs[k], np.float32))
        for k in ("bq", "bk", "bv", "bo", "gamma", "beta")
    }
    for k in ("wq", "wk", "wv", "wo"):
        common[k + "t"] = np.ascontiguousarray(np.asarray(inputs[k], np.float32).T)
    in_maps = []
    for core in range(NCORES):
        b, h = core // 2, core % 2
        loc = x[b][:, h * L : (h + 1) * L]
        oth = x[b][:, (1 - h) * L : (2 - h) * L]
        xf_rot = np.ascontiguousarray(np.concatenate([loc, oth], axis=1))
        m = dict(common)
        m["xf"] = xf_rot
        in_maps.append(m)
    return in_maps


def kernel(**inputs) -> np.ndarray:
    from concourse.bass_utils import run_bass_kernel_spmd

    nc = get_program()
    in_maps = make_in_maps(inputs)
    res = run_bass_kernel_spmd(nc, in_maps, list(range(NCORES)))
    out = np.empty((B, C, HW), np.float32)
    for core in range(NCORES):
        b, h = core // 2, core % 2
        out[b][:, h * L : (h + 1) * L] = res.results[core]["y"]
    return out.reshape(B, C, 64, 64)
